# revision 18
# baseline (speedup 1.0000x reference)
"""Trainium2 Bass kernel for a 2-layer LIF spiking network (DSQN forward).

Math (per batch b, feature h, timestep t; THR=1, snntorch reset='subtract'):
    cur1 = W1 @ x_t + b1                      (precomputable, no recurrence)
    mem1 <- beta1*mem1 + cur1 - H(mem1 - 1)   (reset uses PREVIOUS mem)
    spk1 = H(mem1 - 1)
    cur2 = W2 @ spk1 + b2
    mem2 <- beta2*mem2 + cur2 - H(mem2 - 1)
    spk2 = H(mem2 - 1)
    out_t = W3 @ spk2 + b3

Mapping:
  - Pure data parallel: batch 512 -> 64 per core across 8 cores.
  - Feature-major layout on chip: partitions = H (128), free = (t, b) columns.
  - Host pre-transposes state to (F, T, B64) per core so every DMA is
    contiguous; output comes back as (A, T, B64) and is untransposed on host.
  - One fused custom DVE op does a whole LIF membrane update per step:
        mem_new = mem*beta + cur' - (mem > 1)
    (beta per-partition scalar; bias pre-folded into cur'). 2 DVE ops per
    timestep total (layer-1 + layer-2 interleaved to hide pipeline drain).
  - Spikes are extracted per chunk on the DVE as exact 0/1 bf16 via
    tensor_scalar is_gt(mem, 1) (gets the 2x_2p perf mode); W2/W3 stay
    unfolded and the biases fold into the ACT PSUM->SBUF copies
    (activation Identity with per-partition bias). The Pool engine is
    deliberately idle: its Q7 software ops measured ~8x the cost-model
    estimate. DMA cannot read PSUM and neither can Pool, so all three
    PSUM evacuations (cur1, cur2, out) live on ACT.
  - W2@spk runs as two bf16 matmuls with an exact hi+lo weight split
    (spikes are exact in bf16, so only weight representation matters:
    ~1e-7 per weight). W1@x stays fp32 (splitting x into bf16 hi+lo
    measured 2x the spike-flip error). W3 is plain bf16.
  - Cost model: DVE 298us (2048 LIF + 128 is_gt), ACT 267us (3 copies),
    PE 201us, DMA 106us; measured ~195us wall on hardware.
"""

import numpy as np
import ml_dtypes

import concourse.bacc as bacc
import concourse.mybir as mybir
import concourse.tile as tile
from concourse.bass_utils import run_bass_kernel_spmd
from concourse.dve_spec import Spec, Src0, Src1, C0, C1, One, lower
from concourse.dve_uop import DveOpSpec
from concourse.dve_ops import DveOp, OPS, _CUSTOM_DVE_ROW_BASE, _SUB_OPCODE_FOR_NAME

F32 = mybir.dt.float32
F32R = mybir.dt.float32r
BF16 = mybir.dt.bfloat16
AF = mybir.ActivationFunctionType
ALU = mybir.AluOpType

N_CORES = 8
H = 128
F = 128
A = 16
B_LOC = 64          # batch per core
TC = 16             # timesteps per chunk
COLS = TC * B_LOC   # 1024 columns per chunk
PACK = 4            # W3-output chunks packed per PSUM tile (32-part stride)



def _register_lif_op() -> DveOp:
    """mem_new = Src0*C0 + (Src1 + C1) - (Src0 > 1)."""
    name = "LIF_STEP_ANT"
    for o in OPS:
        if o.name == name:
            return o
    body = Src0 * C0 + (Src1 + C1) - (Src0 > One)
    spec = Spec(
        body=body,
        reference=lambda in0, in1, s0, s1: in0 * s0 + in1 + s1
        - (in0 > 1.0).astype(np.float32),
    )
    shas = {
        ver: DveOpSpec(name=name, uops=lower(spec, ver=ver), rd1_en=True).sha(ver)
        for ver in ("v3", "v4")
    }
    op = DveOp(name, spec, subdim=False, uops_sha=shas)
    OPS.append(op)
    _SUB_OPCODE_FOR_NAME[name] = _CUSTOM_DVE_ROW_BASE + len(OPS) - 1
    return op


def build_program(T: int = 1024):
    """Build the per-core SPMD Bass program. Returns compiled Bacc."""
    assert T % TC == 0
    n_chunks = T // TC
    lif = _register_lif_op()

    nc = bacc.Bacc("TRN2", target_bir_lowering=False, debug=False,
                   num_devices=N_CORES, disable_frame_to_traceback=True)

    x_d = nc.dram_tensor("x", (F, T, B_LOC), F32, kind="ExternalInput")
    m0_d = nc.dram_tensor("mem0", (2, H, B_LOC), F32, kind="ExternalInput")
    w1_d = nc.dram_tensor("w1t", (F, H), F32, kind="ExternalInput")
    w2h_d = nc.dram_tensor("w2h", (H, H), BF16, kind="ExternalInput")
    w2l_d = nc.dram_tensor("w2l", (H, H), BF16, kind="ExternalInput")
    w3_d = nc.dram_tensor("w3t", (H, A), BF16, kind="ExternalInput")
    beta1_d = nc.dram_tensor("beta1", (H, 1), F32, kind="ExternalInput")
    beta2_d = nc.dram_tensor("beta2", (H, 1), F32, kind="ExternalInput")
    c1_d = nc.dram_tensor("c1", (H, 1), F32, kind="ExternalInput")
    c2_d = nc.dram_tensor("c2", (H, 2), F32, kind="ExternalInput")
    c3_d = nc.dram_tensor("c3", (128, 2), F32, kind="ExternalInput")
    out_d = nc.dram_tensor("out", (A, T, B_LOC), F32, kind="ExternalOutput")


    with tile.TileContext(nc) as tc:
        with (
            tc.tile_pool(name="consts", bufs=1) as cpool,
            tc.tile_pool(name="xin", bufs=4) as xpool,
            tc.tile_pool(name="ps1", bufs=2, space="PSUM") as ps1pool,
            tc.tile_pool(name="cur1", bufs=4) as c1pool,
            tc.tile_pool(name="mema", bufs=3) as mapool,
            tc.tile_pool(name="sgn1", bufs=3) as s1pool,
            tc.tile_pool(name="ps2", bufs=1, space="PSUM") as ps2pool,
            tc.tile_pool(name="cur2", bufs=4) as c2pool,
            tc.tile_pool(name="memb", bufs=3) as mbpool,
            tc.tile_pool(name="sgn2", bufs=3) as s2pool,
            tc.tile_pool(name="ps3", bufs=1, space="PSUM") as ps3pool,
            tc.tile_pool(name="outs", bufs=4) as opool,
        ):
            w1_s = cpool.tile([F, H], F32)
            nc.sync.dma_start(w1_s[:], w1_d.ap())
            w2h_s = cpool.tile([H, H], BF16)
            nc.sync.dma_start(w2h_s[:], w2h_d.ap())
            w2l_s = cpool.tile([H, H], BF16)
            nc.sync.dma_start(w2l_s[:], w2l_d.ap())
            w3_s = cpool.tile([H, A], BF16)
            nc.sync.dma_start(w3_s[:], w3_d.ap())
            beta1_s = cpool.tile([H, 1], F32)
            nc.sync.dma_start(beta1_s[:], beta1_d.ap())
            beta2_s = cpool.tile([H, 1], F32)
            nc.sync.dma_start(beta2_s[:], beta2_d.ap())
            c1_s = cpool.tile([H, 1], F32)
            nc.sync.dma_start(c1_s[:], c1_d.ap())
            c2_s = cpool.tile([H, 2], F32)
            nc.sync.dma_start(c2_s[:], c2_d.ap())
            c3_s = cpool.tile([128, 2], F32)
            nc.sync.dma_start(c3_s[:], c3_d.ap())
            m1_s = cpool.tile([H, B_LOC], F32)
            nc.sync.dma_start(m1_s[:], m0_d.ap()[0])
            m2_s = cpool.tile([H, B_LOC], F32)
            nc.sync.dma_start(m2_s[:], m0_d.ap()[1])
            neg1_s = cpool.tile([H, 1], F32)
            nc.vector.memset(neg1_s[:], -1.0)

            prev_a = m1_s[:]
            prev_b = m2_s[:]

            # Layer-2 (B) work runs LAG chunks behind layer-1 (A), and the
            # two LIF chains are interleaved op-by-op on the DVE so
            # consecutive DVE instructions never form a RAW chain (hides
            # the per-op pipeline drain).
            LAG = 2
            cur2_q = {}   # chunk -> cur2 sbuf tile
            mb_q = {}     # chunk -> layer2 mem tile
            cur1_q = {}   # chunk -> cur1 sbuf tile

            def produce_cur1(cc):
                t0 = cc * TC
                x_t = xpool.tile([F, COLS], F32)
                nc.sync.dma_start(
                    x_t[:],
                    x_d.ap()[:, t0:t0 + TC, :].rearrange("p a b -> p (a b)"),
                )
                ps1 = ps1pool.tile([H, COLS], F32)
                for h0 in range(0, COLS, 512):
                    sl = slice(h0, h0 + 512)
                    nc.tensor.matmul(ps1[:, sl], w1_s[:], x_t[:, sl],
                                     start=True, stop=True)
                cur1 = c1pool.tile([H, COLS], F32)
                nc.scalar.activation(cur1[:], ps1[:], AF.Identity,
                                     bias=c1_s[:, 0:1])
                cur1_q[cc] = cur1

            # cur1 is produced two chunks ahead of its LIF consumer so the
            # x-DMA -> W1 -> ACT-copy chain never gates the DVE.
            produce_cur1(0)
            produce_cur1(1)

            for c in range(n_chunks + LAG):
                ca = c            # layer-1 chunk being produced
                cb = c - LAG      # layer-2 chunk being produced
                if ca + 2 < n_chunks:
                    produce_cur1(ca + 2)
                if ca < n_chunks:
                    cur1 = cur1_q.pop(ca)
                    ma = mapool.tile([H, COLS], F32)
                if cb >= 0:
                    mb = mbpool.tile([H, COLS], F32)
                    mb_q[cb] = mb
                    cur2_b = cur2_q.pop(cb)

                for i in range(TC):
                    if ca < n_chunks:
                        src0 = prev_a if i == 0 else ma[:, (i - 1) * B_LOC:i * B_LOC]
                        nc.vector._custom_dve(
                            lif,
                            out=ma[:, i * B_LOC:(i + 1) * B_LOC],
                            in0=src0,
                            in1=cur1[:, i * B_LOC:(i + 1) * B_LOC],
                            s0=beta1_s[:, 0:1],
                            s1=0.0,
                        )
                    if cb >= 0:
                        src0 = prev_b if i == 0 else mb[:, (i - 1) * B_LOC:i * B_LOC]
                        nc.vector._custom_dve(
                            lif,
                            out=mb[:, i * B_LOC:(i + 1) * B_LOC],
                            in0=src0,
                            in1=cur2_b[:, i * B_LOC:(i + 1) * B_LOC],
                            s0=beta2_s[:, 0:1],
                            s1=0.0,
                        )
                if ca < n_chunks:
                    prev_a = ma[:, (TC - 1) * B_LOC:TC * B_LOC]
                if cb >= 0:
                    prev_b = mb[:, (TC - 1) * B_LOC:TC * B_LOC]

                if ca < n_chunks:
                    # Layer-1 spike pass, balanced across engines: most
                    # chunks run sigma = sign(mem-1) in {-1,+1} on ACT
                    # (bias c2 col 0 folds b2 + rowsum(W2)/2); every 4th
                    # chunk runs e = 2*(mem>1) in {0,2} on the DVE (bias
                    # col 1 = plain b2). Both encodings share the halved
                    # W2 hi/lo weights exactly.
                    sg1 = s1pool.tile([H, COLS], BF16)
                    on_pool = (ca % 4 == 3)
                    if on_pool:
                        # e = 2*(mem>1) in {0,2} on the idle GPSIMD engine;
                        # pairs with the plain-b2 bias column.
                        nc.gpsimd.tensor_scalar(sg1[:], ma[:], 1.0, 2.0,
                                                op0=ALU.is_gt, op1=ALU.mult)
                    else:
                        nc.scalar.activation(sg1[:], ma[:], AF.Sign,
                                             bias=neg1_s[:, 0:1])
                    ps2 = ps2pool.tile([H, COLS], F32)
                    for h0 in range(0, COLS, 512):
                        sl = slice(h0, h0 + 512)
                        nc.tensor.matmul(ps2[:, sl], w2h_s[:], sg1[:, sl],
                                         start=True, stop=False)
                        nc.tensor.matmul(ps2[:, sl], w2l_s[:], sg1[:, sl],
                                         start=False, stop=True)
                    cur2 = c2pool.tile([H, COLS], F32)
                    bias_col = c2_s[:, 1:2] if on_pool else c2_s[:, 0:1]
                    nc.scalar.activation(cur2[:], ps2[:], AF.Identity,
                                         bias=bias_col)
                    cur2_q[ca] = cur2

                if cb >= 0:
                    mb = mb_q.pop(cb)
                    # sigma = sign(mem2 - 1) in {-1,0,+1} on the ACT engine
                    # (keeps the spike pass off the saturated DVE). W3 is
                    # halved host-side and rowsum(W3)/2 is folded into c3,
                    # so (W3/2)@sigma + c3' == W3@H(mem2-1) + b3 exactly.
                    sg2 = s2pool.tile([H, COLS], BF16)
                    grp_pool = (cb // PACK) % 2 == 1
                    if grp_pool:
                        nc.gpsimd.tensor_scalar(sg2[:], mb[:], 1.0, 2.0,
                                                op0=ALU.is_gt, op1=ALU.mult)
                    else:
                        nc.scalar.activation(sg2[:], mb[:], AF.Sign,
                                             bias=neg1_s[:, 0:1])
                    # W3 outputs of PACK consecutive chunks land at
                    # 32-partition-aligned offsets of one PSUM tile (the PE
                    # allows tile_position col in {0,32,64,96} for <=32-row
                    # outputs), so a single ACT op evacuates PACK chunks.
                    g = cb % PACK
                    if g == 0:
                        ps3 = ps3pool.tile([32 * PACK, COLS], F32)
                        ps3_cur = ps3
                    else:
                        ps3 = ps3_cur
                    p0 = g * 32
                    for h0 in range(0, COLS, 512):
                        nc.tensor.matmul(ps3[p0:p0 + A, h0:h0 + 512],
                                         w3_s[:], sg2[:, h0:h0 + 512],
                                         start=True, stop=True,
                                         tile_position=(0, p0))
                    if g == PACK - 1:
                        out_t = opool.tile([32 * PACK, COLS], F32)
                        c3col = c3_s[:, 1:2] if grp_pool else c3_s[:, 0:1]
                        nc.scalar.activation(out_t[:], ps3[:], AF.Identity,
                                             bias=c3col)
                        tb0 = (cb - PACK + 1) * TC
                        for gg in range(PACK):
                            nc.sync.dma_start(
                                out_d.ap()[:, tb0 + gg * TC:
                                           tb0 + (gg + 1) * TC, :].rearrange(
                                    "p a b -> p (a b)"),
                                out_t[32 * gg:32 * gg + A, :],
                            )

    nc.compile()
    # Make the serialized BIR independent of kernel.py's on-disk path so
    # the neuron compile cache hits across working directories.
    fixed_dbg = mybir.OpDebugInfo(filename="kernel.py", lineno=0,
                                  kernel_name="build_program:")
    _dbg_cache = {}

    def _sanitize(dbg):
        if dbg is None:
            return None
        key = (dbg.op_name, dbg.lineno, dbg.bass_funcname, dbg.kernel_name,
               dbg.ant_layer, dbg.ant_annotation)
        if key not in _dbg_cache:
            _dbg_cache[key] = mybir.OpDebugInfo(
                op_name=dbg.op_name, filename="kernel.py", lineno=dbg.lineno,
                bass_funcname=dbg.bass_funcname, kernel_name=dbg.kernel_name,
                ant_layer=dbg.ant_layer, ant_annotation=dbg.ant_annotation)
        return _dbg_cache[key]

    for fn in nc.m.functions:
        for alloc in fn.allocations:
            for ml in (getattr(alloc, "memorylocations", None) or []):
                if getattr(ml, "ant_debug", None) is not None:
                    ml.ant_debug = fixed_dbg
        for blk in fn.blocks:
            for inst in blk.instructions:
                inst.debug = _sanitize(inst.debug)
    return nc


def make_in_maps(state_batch, hidden_states, W1, b1, beta1, W2, b2, beta2,
                 W3, b3, T=None):
    """Host-side prep: shard/transpose per core, fold constants (exact)."""
    x = np.asarray(state_batch, np.float32)
    hs = np.asarray(hidden_states, np.float32)
    B = x.shape[0]
    if T is None:
        T = x.shape[1]
    W1 = np.asarray(W1, np.float32)
    W2 = np.asarray(W2, np.float32)
    W3 = np.asarray(W3, np.float32)

    def split_hl(a):
        hi = a.astype(ml_dtypes.bfloat16)
        lo = (a - hi.astype(np.float32)).astype(ml_dtypes.bfloat16)
        return hi, lo

    w1t = np.ascontiguousarray(W1.T)
    # Layer-1 spikes arrive sigma-encoded (sign(mem-1) in {-1,+1}):
    # W2@spk == (W2/2)@sigma + rowsum(W2)/2. The hi/lo split is halved
    # exactly (exponent shift); the rowsum correction uses the actual
    # split values so it matches what the PE accumulates.
    w2h_f, w2l_f = split_hl(np.ascontiguousarray(W2.T))
    w2h = (w2h_f.astype(np.float32) * 0.5).astype(ml_dtypes.bfloat16)
    w2l = (w2l_f.astype(np.float32) * 0.5).astype(ml_dtypes.bfloat16)
    w2_eff = w2h_f.astype(np.float32) + w2l_f.astype(np.float32)  # (F=h_in, H=h_out)
    # Layer-2 spikes arrive sigma-encoded (sign(mem-1) in {-1,+1}):
    # W3@spk == (W3/2)@sigma + rowsum(W3)/2, folded into the out bias.
    w3t = (np.ascontiguousarray(W3.T).astype(ml_dtypes.bfloat16)
           .astype(np.float32) * 0.5).astype(ml_dtypes.bfloat16)
    be1 = np.clip(np.asarray(beta1, np.float32), 0.0, 1.0).reshape(H, 1)
    be2 = np.clip(np.asarray(beta2, np.float32), 0.0, 1.0).reshape(H, 1)
    c1 = np.asarray(b1, np.float32).reshape(H, 1)
    b2f = np.asarray(b2, np.float32)
    c2 = np.stack([b2f + 0.5 * w2_eff.sum(axis=0), b2f], axis=1)  # (H, 2)
    w3bf = np.asarray(W3, np.float32).astype(ml_dtypes.bfloat16)
    b3f = np.asarray(b3, np.float32)
    c3_sig = (b3f + 0.5 * w3bf.astype(np.float32).sum(axis=1)).reshape(A, 1)
    c3_pln = b3f.reshape(A, 1)
    c3 = np.zeros((128, 2), np.float32)
    for g in range(4):
        c3[32 * g:32 * g + A, 0:1] = c3_sig
        c3[32 * g:32 * g + A, 1:2] = c3_pln

    in_maps = []
    for c in range(N_CORES):
        bs = slice(c * B_LOC, (c + 1) * B_LOC)
        xc = np.ascontiguousarray(x[bs, :T].transpose(2, 1, 0))      # (F,T,B)
        m0 = np.ascontiguousarray(hs[bs, 0].transpose(1, 2, 0))      # (2,H,B)
        in_maps.append({
            "x": xc, "mem0": m0, "w1t": w1t,
            "w2h": w2h, "w2l": w2l, "w3t": w3t,
            "beta1": be1, "beta2": be2, "c1": c1, "c2": c2, "c3": c3,
        })
    return in_maps


def assemble_output(results, B, T):
    out = np.empty((B, T, A), np.float32)
    for c in range(len(results)):
        bs = slice(c * B_LOC, (c + 1) * B_LOC)
        out[bs] = results[c]["out"].transpose(2, 1, 0)               # (B,T,A)
    return out


_NC_CACHE = {}


def kernel(**inputs) -> np.ndarray:
    x = np.asarray(inputs["state_batch"], np.float32)
    B, T, _ = x.shape
    if T not in _NC_CACHE:
        _NC_CACHE[T] = build_program(T)
    nc = _NC_CACHE[T]
    in_maps = make_in_maps(**inputs, T=T)
    res = run_bass_kernel_spmd(nc, in_maps, core_ids=list(range(N_CORES)),
                               trace=False)
    return assemble_output(res.results, B, T)



# revision 30
# speedup vs baseline: 5.3675x; 5.3675x over previous
"""Trainium2 Bass kernel for a 2-layer LIF spiking network (DSQN forward).

Math (per batch b, feature h, timestep t; THR=1, snntorch reset='subtract'):
    cur1 = W1 @ x_t + b1                      (precomputable, no recurrence)
    mem1 <- beta1*mem1 + cur1 - H(mem1 - 1)   (reset uses PREVIOUS mem)
    spk1 = H(mem1 - 1)
    cur2 = W2 @ spk1 + b2
    mem2 <- beta2*mem2 + cur2 - H(mem2 - 1)
    spk2 = H(mem2 - 1)
    out_t = W3 @ spk2 + b3

Mapping:
  - Pure data parallel: batch 512 -> 64 per core across 8 cores.
  - Feature-major layout on chip: partitions = H (128), free = (t, b) columns.
  - Host pre-transposes state to (F, T, B64) per core so every DMA is
    contiguous; output comes back as (A, T, B64) and is untransposed on host.
  - One fused custom DVE op does a whole LIF membrane update per step:
        mem_new = mem*beta + cur' - (mem > 1)
    (beta per-partition scalar; bias pre-folded into cur'). 2 DVE ops per
    timestep total (layer-1 + layer-2 interleaved to hide pipeline drain).
  - Spikes are extracted per chunk on the DVE as exact 0/1 bf16 via
    tensor_scalar is_gt(mem, 1) (gets the 2x_2p perf mode); W2/W3 stay
    unfolded and the biases fold into the ACT PSUM->SBUF copies
    (activation Identity with per-partition bias). The Pool engine is
    deliberately idle: its Q7 software ops measured ~8x the cost-model
    estimate. DMA cannot read PSUM and neither can Pool, so all three
    PSUM evacuations (cur1, cur2, out) live on ACT.
  - W2@spk runs as two bf16 matmuls with an exact hi+lo weight split
    (spikes are exact in bf16, so only weight representation matters:
    ~1e-7 per weight). W1@x stays fp32 (splitting x into bf16 hi+lo
    measured 2x the spike-flip error). W3 is plain bf16.
  - Cost model: DVE 298us (2048 LIF + 128 is_gt), ACT 267us (3 copies),
    PE 201us, DMA 106us; measured ~195us wall on hardware.
"""

import numpy as np
import ml_dtypes

import concourse.bacc as bacc
import concourse.mybir as mybir
import concourse.tile as tile
from concourse.bass_utils import run_bass_kernel_spmd
from concourse.dve_spec import Spec, Src0, Src1, C0, C1, One, lower
from concourse.dve_uop import DveOpSpec
from concourse.dve_ops import DveOp, OPS, _CUSTOM_DVE_ROW_BASE, _SUB_OPCODE_FOR_NAME

F32 = mybir.dt.float32
F32R = mybir.dt.float32r
BF16 = mybir.dt.bfloat16
AF = mybir.ActivationFunctionType
ALU = mybir.AluOpType

N_CORES = 8
H = 128
F = 128
A = 16
B_LOC = 64          # batch per core
TC = 16             # timesteps per chunk
COLS = TC * B_LOC   # 1024 columns per chunk
PACK = 4            # W3-output chunks packed per PSUM tile (32-part stride)



def _register_lif_op() -> DveOp:
    """mem_new = Src0*C0 + (Src1 + C1) - (Src0 > 1)."""
    name = "LIF_STEP_ANT"
    for o in OPS:
        if o.name == name:
            return o
    body = Src0 * C0 + (Src1 + C1) - (Src0 > One)
    spec = Spec(
        body=body,
        reference=lambda in0, in1, s0, s1: in0 * s0 + in1 + s1
        - (in0 > 1.0).astype(np.float32),
    )
    shas = {
        ver: DveOpSpec(name=name, uops=lower(spec, ver=ver), rd1_en=True).sha(ver)
        for ver in ("v3", "v4")
    }
    op = DveOp(name, spec, subdim=False, uops_sha=shas)
    OPS.append(op)
    _SUB_OPCODE_FOR_NAME[name] = _CUSTOM_DVE_ROW_BASE + len(OPS) - 1
    return op


def build_program(T: int = 1024):
    """Build the per-core SPMD Bass program. Returns compiled Bacc."""
    assert T % TC == 0
    n_chunks = T // TC
    lif = _register_lif_op()

    nc = bacc.Bacc("TRN2", target_bir_lowering=False, debug=False,
                   num_devices=N_CORES, disable_frame_to_traceback=True)

    x_d = nc.dram_tensor("x", (F, T, B_LOC), F32, kind="ExternalInput")
    # All f32 constants ride ONE DMA (each dma_start costs ~600ns of
    # SP sequencer issue time at startup, and every extra input tensor
    # adds per-call dispatch overhead): columns are
    # [w1t(128), beta1, beta2, c1, c2(2), c3(2), m1(64), m2(64)] = 263.
    cp_d = nc.dram_tensor("cpack", (128, 263), F32, kind="ExternalInput")
    # bf16 weights likewise: [w2h(128), w2l(128), w3t(16)] = 272 columns.
    wp_d = nc.dram_tensor("wpack", (H, 2 * H + A), BF16, kind="ExternalInput")
    out_d = nc.dram_tensor("out", (A, T, B_LOC), F32, kind="ExternalOutput")


    with tile.TileContext(nc) as tc:
        with (
            tc.tile_pool(name="consts", bufs=1) as cpool,
            tc.tile_pool(name="xin", bufs=5) as xpool,
            tc.tile_pool(name="ps1", bufs=2, space="PSUM") as ps1pool,
            tc.tile_pool(name="cur1", bufs=5) as c1pool,
            tc.tile_pool(name="mema", bufs=4) as mapool,
            tc.tile_pool(name="sgn1", bufs=4) as s1pool,
            tc.tile_pool(name="ps2", bufs=1, space="PSUM") as ps2pool,
            tc.tile_pool(name="cur2", bufs=5) as c2pool,
            tc.tile_pool(name="memb", bufs=4) as mbpool,
            tc.tile_pool(name="sgn2", bufs=4) as s2pool,
            tc.tile_pool(name="ps3", bufs=1, space="PSUM") as ps3pool,
            tc.tile_pool(name="outs", bufs=4) as opool,
        ):
            cpack = cpool.tile([128, 263], F32)
            nc.sync.dma_start(cpack[:], cp_d.ap())
            w1_s = cpack[:, 0:H]
            wpack = cpool.tile([H, 2 * H + A], BF16)
            nc.sync.dma_start(wpack[:], wp_d.ap())
            beta1_s = cpack[:, H + 0:H + 1]
            beta2_s = cpack[:, H + 1:H + 2]
            c1_s = cpack[:, H + 2:H + 3]
            c2_sig = cpack[:, H + 3:H + 4]
            c2_pln = cpack[:, H + 4:H + 5]
            c3_s = cpack[:, H + 5:H + 6]
            m1_s = cpack[:, H + 7:H + 7 + B_LOC]
            m2_s = cpack[:, H + 7 + B_LOC:H + 7 + 2 * B_LOC]
            w2h_s = wpack[:, 0:H]
            w2l_s = wpack[:, H:2 * H]
            w3_s = wpack[:, 2 * H:2 * H + A]
            neg1_s = cpool.tile([H, 1], F32)
            nc.vector.memset(neg1_s[:], -1.0)

            # Startup priming: absorb the one-time ACT table load while
            # the first x-chunk DMA is still in flight.
            warm = cpool.tile([H, 8], F32)
            nc.vector.memset(warm[:], 0.0)
            nc.scalar.activation(warm[:], warm[:], AF.Sign,
                                 bias=neg1_s[:, 0:1])

            prev_a = m1_s
            prev_b = m2_s

            # Layer-2 (B) work runs LAG chunks behind layer-1 (A), and the
            # two LIF chains are interleaved op-by-op on the DVE so
            # consecutive DVE instructions never form a RAW chain (hides
            # the per-op pipeline drain).
            LAG = 3
            cur2_q = {}   # chunk -> cur2 sbuf tile
            mb_q = {}     # chunk -> layer2 mem tile
            cur1_q = {}   # chunk -> cur1 sbuf tile

            def produce_cur1(cc, evac_slices=1):
                t0 = cc * TC
                x_t = xpool.tile([F, COLS], F32)
                nc.sync.dma_start(
                    x_t[:],
                    x_d.ap()[:, t0:t0 + TC, :].rearrange("p a b -> p (a b)"),
                )
                ps1 = ps1pool.tile([H, COLS], F32)
                mm_step = 512 // evac_slices
                for h0 in range(0, COLS, mm_step):
                    sl = slice(h0, h0 + mm_step)
                    nc.tensor.matmul(ps1[:, sl], w1_s, x_t[:, sl],
                                     start=True, stop=True)
                cur1 = c1pool.tile([H, COLS], F32)
                step = COLS // evac_slices
                for e0 in range(0, COLS, step):
                    nc.scalar.activation(cur1[:, e0:e0 + step],
                                         ps1[:, e0:e0 + step], AF.Identity,
                                         bias=c1_s)
                cur1_q[cc] = cur1

            # cur1 is produced two chunks ahead of its LIF consumer so the
            # x-DMA -> W1 -> ACT-copy chain never gates the DVE.
            produce_cur1(0, evac_slices=4)
            produce_cur1(1, evac_slices=2)

            for c in range(n_chunks + LAG):
                ca = c            # layer-1 chunk being produced
                cb = c - LAG      # layer-2 chunk being produced
                if ca + 2 < n_chunks:
                    produce_cur1(ca + 2)
                if ca < n_chunks:
                    cur1 = cur1_q.pop(ca)
                    ma = mapool.tile([H, COLS], F32)
                if cb >= 0:
                    mb = mbpool.tile([H, COLS], F32)
                    mb_q[cb] = mb
                    cur2_b = cur2_q.pop(cb)

                for i in range(TC):
                    if ca < n_chunks:
                        src0 = prev_a if i == 0 else ma[:, (i - 1) * B_LOC:i * B_LOC]
                        nc.vector._custom_dve(
                            lif,
                            out=ma[:, i * B_LOC:(i + 1) * B_LOC],
                            in0=src0,
                            in1=cur1[:, i * B_LOC:(i + 1) * B_LOC],
                            s0=beta1_s,
                            s1=0.0,
                        )
                    if cb >= 0:
                        src0 = prev_b if i == 0 else mb[:, (i - 1) * B_LOC:i * B_LOC]
                        nc.vector._custom_dve(
                            lif,
                            out=mb[:, i * B_LOC:(i + 1) * B_LOC],
                            in0=src0,
                            in1=cur2_b[:, i * B_LOC:(i + 1) * B_LOC],
                            s0=beta2_s,
                            s1=0.0,
                        )
                if ca < n_chunks:
                    prev_a = ma[:, (TC - 1) * B_LOC:TC * B_LOC]
                if cb >= 0:
                    prev_b = mb[:, (TC - 1) * B_LOC:TC * B_LOC]

                if ca < n_chunks:
                    # Layer-1 spike pass, balanced across engines: most
                    # chunks run sigma = sign(mem-1) in {-1,+1} on ACT
                    # (bias c2 col 0 folds b2 + rowsum(W2)/2); every 4th
                    # chunk runs e = 2*(mem>1) in {0,2} on the DVE (bias
                    # col 1 = plain b2). Both encodings share the halved
                    # W2 hi/lo weights exactly.
                    sg1 = s1pool.tile([H, COLS], BF16)
                    # e = 2*(mem>1) in {0,2} on the DVE for every 4th chunk
                    # (engine balance); sigma = sign(mem-1) on ACT otherwise.
                    # GPSIMD was tried here and measured ~8x the cost-model
                    # estimate on hardware -- do not offload spikes to Pool.
                    on_dve = (ca % 4 == 3)
                    if on_dve:
                        nc.vector.tensor_scalar(sg1[:], ma[:], 1.0, 2.0,
                                                op0=ALU.is_gt, op1=ALU.mult)
                    else:
                        nc.scalar.activation(sg1[:], ma[:], AF.Sign,
                                             bias=neg1_s[:, 0:1])
                    ps2 = ps2pool.tile([H, COLS], F32)
                    for h0 in range(0, COLS, 512):
                        sl = slice(h0, h0 + 512)
                        nc.tensor.matmul(ps2[:, sl], w2h_s, sg1[:, sl],
                                         start=True, stop=False)
                        nc.tensor.matmul(ps2[:, sl], w2l_s, sg1[:, sl],
                                         start=False, stop=True)
                    cur2 = c2pool.tile([H, COLS], F32)
                    bias_col = c2_pln if on_dve else c2_sig
                    nc.scalar.activation(cur2[:], ps2[:], AF.Identity,
                                         bias=bias_col)
                    cur2_q[ca] = cur2

                if cb >= 0:
                    mb = mb_q.pop(cb)
                    # sigma = sign(mem2 - 1) in {-1,0,+1} on the ACT engine
                    # (keeps the spike pass off the saturated DVE). W3 is
                    # halved host-side and rowsum(W3)/2 is folded into c3,
                    # so (W3/2)@sigma + c3' == W3@H(mem2-1) + b3 exactly.
                    sg2 = s2pool.tile([H, COLS], BF16)
                    nc.scalar.activation(sg2[:], mb[:], AF.Sign,
                                         bias=neg1_s[:, 0:1])
                    # W3 outputs of PACK consecutive chunks land at
                    # 32-partition-aligned offsets of one PSUM tile (the PE
                    # allows tile_position col in {0,32,64,96} for <=32-row
                    # outputs), so a single ACT op evacuates PACK chunks.
                    g = cb % PACK
                    if g == 0:
                        ps3 = ps3pool.tile([32 * PACK, COLS], F32)
                        ps3_cur = ps3
                    else:
                        ps3 = ps3_cur
                    p0 = g * 32
                    for h0 in range(0, COLS, 512):
                        nc.tensor.matmul(ps3[p0:p0 + A, h0:h0 + 512],
                                         w3_s, sg2[:, h0:h0 + 512],
                                         start=True, stop=True,
                                         tile_position=(0, p0))
                    if g == PACK - 1:
                        out_t = opool.tile([32 * PACK, COLS], F32)
                        nc.scalar.activation(out_t[:], ps3[:], AF.Identity,
                                             bias=c3_s)
                        tb0 = (cb - PACK + 1) * TC
                        for gg in range(PACK):
                            nc.sync.dma_start(
                                out_d.ap()[:, tb0 + gg * TC:
                                           tb0 + (gg + 1) * TC, :].rearrange(
                                    "p a b -> p (a b)"),
                                out_t[32 * gg:32 * gg + A, :],
                            )

    nc.compile()
    # Make the serialized BIR independent of kernel.py's on-disk path so
    # the neuron compile cache hits across working directories.
    fixed_dbg = mybir.OpDebugInfo(filename="kernel.py", lineno=0,
                                  kernel_name="build_program:")
    _dbg_cache = {}

    def _sanitize(dbg):
        if dbg is None:
            return None
        key = (dbg.op_name, dbg.lineno, dbg.bass_funcname, dbg.kernel_name,
               dbg.ant_layer, dbg.ant_annotation)
        if key not in _dbg_cache:
            _dbg_cache[key] = mybir.OpDebugInfo(
                op_name=dbg.op_name, filename="kernel.py", lineno=dbg.lineno,
                bass_funcname=dbg.bass_funcname, kernel_name=dbg.kernel_name,
                ant_layer=dbg.ant_layer, ant_annotation=dbg.ant_annotation)
        return _dbg_cache[key]

    for fn in nc.m.functions:
        for alloc in fn.allocations:
            for ml in (getattr(alloc, "memorylocations", None) or []):
                if getattr(ml, "ant_debug", None) is not None:
                    ml.ant_debug = fixed_dbg
        for blk in fn.blocks:
            for inst in blk.instructions:
                inst.debug = _sanitize(inst.debug)
    return nc


def make_in_maps(state_batch, hidden_states, W1, b1, beta1, W2, b2, beta2,
                 W3, b3, T=None):
    """Host-side prep: shard/transpose per core, fold constants (exact)."""
    x = np.asarray(state_batch, np.float32)
    hs = np.asarray(hidden_states, np.float32)
    B = x.shape[0]
    if T is None:
        T = x.shape[1]
    W1 = np.asarray(W1, np.float32)
    W2 = np.asarray(W2, np.float32)
    W3 = np.asarray(W3, np.float32)

    def split_hl(a):
        hi = a.astype(ml_dtypes.bfloat16)
        lo = (a - hi.astype(np.float32)).astype(ml_dtypes.bfloat16)
        return hi, lo

    w1t = np.ascontiguousarray(W1.T)
    # Layer-1 spikes arrive sigma-encoded (sign(mem-1) in {-1,+1}):
    # W2@spk == (W2/2)@sigma + rowsum(W2)/2. The hi/lo split is halved
    # exactly (exponent shift); the rowsum correction uses the actual
    # split values so it matches what the PE accumulates.
    w2h_f, w2l_f = split_hl(np.ascontiguousarray(W2.T))
    w2h = (w2h_f.astype(np.float32) * 0.5).astype(ml_dtypes.bfloat16)
    w2l = (w2l_f.astype(np.float32) * 0.5).astype(ml_dtypes.bfloat16)
    w2_eff = w2h_f.astype(np.float32) + w2l_f.astype(np.float32)  # (F=h_in, H=h_out)
    # Layer-2 spikes arrive sigma-encoded (sign(mem-1) in {-1,+1}):
    # W3@spk == (W3/2)@sigma + rowsum(W3)/2, folded into the out bias.
    w3t = (np.ascontiguousarray(W3.T).astype(ml_dtypes.bfloat16)
           .astype(np.float32) * 0.5).astype(ml_dtypes.bfloat16)
    be1 = np.clip(np.asarray(beta1, np.float32), 0.0, 1.0).reshape(H, 1)
    be2 = np.clip(np.asarray(beta2, np.float32), 0.0, 1.0).reshape(H, 1)
    c1 = np.asarray(b1, np.float32).reshape(H, 1)
    b2f = np.asarray(b2, np.float32)
    c2 = np.stack([b2f + 0.5 * w2_eff.sum(axis=0), b2f], axis=1)  # (H, 2)
    w3bf = np.asarray(W3, np.float32).astype(ml_dtypes.bfloat16)
    b3f = np.asarray(b3, np.float32)
    c3_sig = (b3f + 0.5 * w3bf.astype(np.float32).sum(axis=1)).reshape(A, 1)
    c3_pln = b3f.reshape(A, 1)
    c3 = np.zeros((128, 1), np.float32)
    for g in range(4):
        c3[32 * g:32 * g + A, 0:1] = c3_sig

    # One bf16 weight pack: [w2h | w2l | w3t] along columns.
    wpack = np.concatenate(
        [w2h.astype(np.float32), w2l.astype(np.float32),
         w3t.astype(np.float32)], axis=1).astype(ml_dtypes.bfloat16)

    in_maps = []
    for c in range(N_CORES):
        bs = slice(c * B_LOC, (c + 1) * B_LOC)
        xc = np.ascontiguousarray(x[bs, :T].transpose(2, 1, 0))      # (F,T,B)
        m0 = hs[bs, 0].transpose(1, 2, 0)                            # (2,H,B)
        # One f32 const pack: [w1t,beta1,beta2,c1,c2sig,c2pln,c3,pad,m1,m2].
        cpack = np.zeros((128, 263), np.float32)
        cpack[:, 0:128] = w1t
        cpack[:, 128:129] = be1
        cpack[:, 129:130] = be2
        cpack[:, 130:131] = c1
        cpack[:, 131:133] = c2
        cpack[:, 133:134] = c3
        cpack[:, 135:135 + B_LOC] = m0[0]
        cpack[:, 135 + B_LOC:135 + 2 * B_LOC] = m0[1]
        in_maps.append({"x": xc, "cpack": cpack, "wpack": wpack})
    return in_maps


def assemble_output(results, B, T):
    out = np.empty((B, T, A), np.float32)
    for c in range(len(results)):
        bs = slice(c * B_LOC, (c + 1) * B_LOC)
        out[bs] = results[c]["out"].transpose(2, 1, 0)               # (B,T,A)
    return out


_NC_CACHE = {}


def kernel(**inputs) -> np.ndarray:
    x = np.asarray(inputs["state_batch"], np.float32)
    B, T, _ = x.shape
    if T not in _NC_CACHE:
        _NC_CACHE[T] = build_program(T)
    nc = _NC_CACHE[T]
    in_maps = make_in_maps(**inputs, T=T)
    res = run_bass_kernel_spmd(nc, in_maps, core_ids=list(range(N_CORES)),
                               trace=False)
    return assemble_output(res.results, B, T)

